# revision 1
# baseline (speedup 1.0000x reference)
"""Bidirectional LSTM Trainium2 Bass kernel.

Problem: T=128, B=128, IN=512, H=512, OUT=512 (fp32 reference).
Sharding: data-parallel over batch + direction-parallel:
  cores 0-3: forward LSTM, batch slices 0:32, 32:64, 64:96, 96:128
  cores 4-7: backward LSTM (time-reversed x), same batch slices
Each core (phases overlap via dependency scheduling; phase-1/3 work is
emitted interleaved into the recurrence so the in-order TensorE stream
fills recurrence stalls):
  phase 1: xw[t] = x[t] @ W_ih.T + (b_ih + b_hh), 4 timesteps per GEMM
           (M=128); bias added by VectorE during PSUM evacuation into an
           8-chunk SBUF ring consumed by phase 2
  phase 2: 128 sequential LSTM steps:
           gates = xw[t] (seeded into PSUM via a column-selection matmul
           against ident128, which also sets PSUM has_written)
                 + h[t-1] @ W_hh.T (4 K-tile matmuls per 512-col bank)
           sigmoid/tanh on ScalarE, cell update on VectorE,
           h transposed on TensorE for the next step's stationary operand.
           All per-step tensors are split into per-half (256-hidden-unit)
           tiles (gates, acts, c, h, and the hT history split by k-pair)
           because Tile tracks dependencies at tile granularity - the
           split lets each half's chain and the next step's matmuls
           overlap.
  phase 3: partial out[t] = h[t] @ W_lin[:, dir*H:(dir+1)*H].T into an
           SBUF buffer, one final DMA to DRAM
Host combines: out = out_fwd + flip_t(out_bwd) + b_lin.

All matmuls run in bf16 (fp32 PSUM accumulation); the cell state c stays
fp32. Gate columns are host-permuted to [o f i g] per 256-wide half so
one sigmoid instruction covers o,f,i contiguously.
"""

import sys

sys.path.insert(0, "/opt/trn_rl_repo")

import functools

import ml_dtypes
import numpy as np

import concourse.bass as bass
import concourse.tile as tile
from concourse import bacc, mybir
from concourse.bass_utils import run_bass_kernel_spmd

T, B, IN, H, OUT = 128, 128, 512, 512, 512
NCORES = 8
BL = B // 4  # batch per core (4 cores per direction)
G4 = 4 * H  # 2048 gate columns
KT = IN // 128  # 4 K-tiles of 128
NB = G4 // 512  # 4 psum banks of 512 gate cols
TCH = T // 4  # 32 chunks of 4 timesteps for phase 1/3
RING = int(__import__("os").environ.get("LSTM_RING", "8"))  # xw ring depth (chunks)

import os

KNOB_LOOKAHEAD = int(os.environ.get("LSTM_LOOKAHEAD", "4"))
KNOB_XPOSE_DMA = os.environ.get("LSTM_XPOSE_DMA", "0") == "1"
KNOB_MERGE_HALVES = os.environ.get("LSTM_MERGE_HALVES", "0") == "1"
KNOB_SPLIT_FI = os.environ.get("LSTM_SPLIT_FI", "0") == "1"
KNOB_COPIES_ACT = os.environ.get("LSTM_COPIES_ACT", "0") == "1"
KNOB_BUFS_ACTS = int(os.environ.get("LSTM_BUFS_ACTS", "2"))
KNOB_BUFS_TMPS = int(os.environ.get("LSTM_BUFS_TMPS", "2"))
KNOB_PST_BUFS = int(os.environ.get("LSTM_PST_BUFS", "1"))

BF16 = mybir.dt.bfloat16
FP32 = mybir.dt.float32
AF = mybir.ActivationFunctionType


def build_nc(reps=1):
    nc = bacc.Bacc(None, target_bir_lowering=False)
    xT = nc.dram_tensor("xT", [128, TCH, KT, 4, BL], BF16, kind="ExternalInput")
    wih = nc.dram_tensor("wih", [128, KT, G4], BF16, kind="ExternalInput")
    whh = nc.dram_tensor("whh", [128, KT, G4], BF16, kind="ExternalInput")
    bias = nc.dram_tensor("biasr", [128, G4], BF16, kind="ExternalInput")
    wlin = nc.dram_tensor("wlin", [128, KT, OUT], BF16, kind="ExternalInput")
    id32 = nc.dram_tensor("id32", [BL, BL], BF16, kind="ExternalInput")
    id128 = nc.dram_tensor("id128", [128, 128], BF16, kind="ExternalInput")
    outp = nc.dram_tensor("outp", [128, TCH, OUT], FP32, kind="ExternalOutput")

    with tile.TileContext(nc) as tc:
        with (
            tc.tile_pool(name="const", bufs=1) as constp,
            tc.tile_pool(name="xwring", bufs=RING) as ringp,
            tc.tile_pool(name="p1x", bufs=4) as p1x,
            tc.tile_pool(name="acts", bufs=KNOB_BUFS_ACTS) as acts_p,
            tc.tile_pool(name="tmps", bufs=KNOB_BUFS_TMPS) as tmps_p,
            tc.tile_pool(name="p1ps", bufs=1, space="PSUM") as p1ps,
            tc.tile_pool(name="ps2", bufs=1, space="PSUM") as ps2,
            tc.tile_pool(name="psT", bufs=1, space="PSUM") as psT,
            tc.tile_pool(name="ps3", bufs=1, space="PSUM") as ps3,
        ):
            id32_sb = constp.tile([BL, BL], BF16)
            nc.sync.dma_start(id32_sb[:], id32[:])
            id128_sb = constp.tile([128, 128], BF16)
            nc.sync.dma_start(id128_sb[:], id128[:])
            wih_sb = constp.tile([128, KT, G4], BF16)
            nc.sync.dma_start(wih_sb[:], wih[:])
            whh_sb = constp.tile([128, KT, G4], BF16)
            nc.sync.dma_start(whh_sb[:], whh[:])
            bias_sb = constp.tile([128, G4], BF16)
            nc.sync.dma_start(bias_sb[:], bias[:])
            wlin_sb = constp.tile([128, KT, OUT], BF16)
            nc.sync.dma_start(wlin_sb[:], wlin[:])
            # h^T history, split by k-pair so the next step's k=0,1 matmuls
            # depend only on the half-0 copy (Tile tracks deps per tile).
            # hT_k[p][:, ch, kk, ti, :] holds h_t[128*(2p+kk):...,:] bf16.
            hT_k = [
                constp.tile([128, TCH, 2, 4, BL], BF16, name=f"hTk{p}")
                for p in range(2)
            ]
            hT0 = constp.tile([128, KT, BL], BF16)
            nc.vector.memset(hT0[:], 0.0)
            # cell state per half (separate tiles -> independent dep chains)
            c_half = [constp.tile([BL, H // 2], FP32, name=f"c{q}") for q in range(2)]
            out_all = constp.tile([128, TCH, OUT], FP32)

            for _rep in range(reps):
                for q in range(2):
                    nc.vector.memset(c_half[q][:], 0.0)
                xw_tiles = []
                n_halves_emitted = [0]

                # ---- phase 1 emitter: xw = x @ W_ih.T + bias, emitted in
                # half-chunk (10-matmul) granules spread across the
                # recurrence so the scheduler can fill PE stalls.
                def emit_xw_half():
                    hidx = n_halves_emitted[0]
                    if hidx >= 2 * TCH:
                        return
                    n_halves_emitted[0] += 1
                    ch, half = hidx // 2, hidx % 2
                    if half == 0:
                        xt = p1x.tile([128, KT, 4, BL], BF16, tag="xt")
                        nc.sync.dma_start(xt[:], xT[:, ch])
                        xwr = ringp.tile([128, G4], BF16, tag="xw")
                        xw_tiles.append((xwr, xt))
                    xwr, xt = xw_tiles[ch]
                    for nb2 in range(2):
                        pxw = p1ps.tile([128, 512], FP32, tag="pxw")
                        cg = slice(
                            1024 * half + 512 * nb2, 1024 * half + 512 * nb2 + 512
                        )
                        for k in range(KT):
                            nc.tensor.matmul(
                                pxw[:],
                                xt[:, k],
                                wih_sb[:, k, cg],
                                start=(k == 0),
                                stop=(k == KT - 1),
                            )
                        nc.vector.tensor_add(xwr[:, cg], pxw[:], bias_sb[:, cg])

                # ---- phase 3 emitter: partial linear out = h @ W_lin_half.T
                def emit_linear_chunk(ch):
                    po = ps3.tile([128, OUT], FP32, tag="po")
                    for k in range(KT):
                        nc.tensor.matmul(
                            po[:],
                            hT_k[k // 2][:, ch, k % 2],
                            wlin_sb[:, k],
                            start=(k == 0),
                            stop=(k == KT - 1),
                        )
                    nc.vector.tensor_copy(out_all[:, ch, :], po[:])

                def emit_seeds(t):
                    # seed PSUM with xw row-block (sets has_written);
                    # one psum tile PER BANK -> finest dep granularity
                    ch, ti = t // 4, t % 4
                    gh = [
                        ps2.tile([BL, 512], FP32, tag=f"gates{b}", name=f"gates{b}")
                        for b in range(4)
                    ]
                    for b in range(4):
                        nc.tensor.matmul(
                            gh[b][:],
                            id128_sb[:, 32 * ti : 32 * ti + 32],
                            xw_tiles[ch][0][:, 512 * b : 512 * b + 512],
                            start=True,
                            stop=False,
                        )
                    return gh

                # prologue: phase-1 lookahead + step 0 seeds
                for _ in range(2 * KNOB_LOOKAHEAD):
                    emit_xw_half()
                gh = emit_seeds(0)

                for t in range(T):
                    ch, ti = t // 4, t % 4
                    # W-MMs: half-0's 8 matmuls first (its activations can
                    # then start mid-block); k ascending within each half so
                    # the previous step's half-1 transpose has slack.
                    for q in range(2):
                        for k in range(KT):
                            hT_prev = (
                                hT0[:, k]
                                if t == 0
                                else hT_k[k // 2][
                                    :, (t - 1) // 4, k % 2, (t - 1) % 4, :
                                ]
                            )
                            for nb2 in range(2):
                                nc.tensor.matmul(
                                    gh[2 * q + nb2][:],
                                    hT_prev,
                                    whh_sb[:, k, 1024 * q + 512 * nb2 :
                                           1024 * q + 512 * nb2 + 512],
                                    start=False,
                                    stop=(k == KT - 1),
                                )
                    ah = [
                        acts_p.tile([BL, 1024], BF16, tag=f"acts{q}", name=f"acts{q}")
                        for q in range(2)
                    ]
                    tct = [
                        tmps_p.tile([BL, 256], BF16, tag=f"tct{q}", name=f"tct{q}")
                        for q in range(2)
                    ]
                    fc = [
                        tmps_p.tile([BL, 256], FP32, tag=f"fc{q}", name=f"fc{q}")
                        for q in range(2)
                    ]
                    ig = [
                        tmps_p.tile([BL, 256], FP32, tag=f"ig{q}", name=f"ig{q}")
                        for q in range(2)
                    ]
                    h_sb = [
                        tmps_p.tile([BL, 256], BF16, tag=f"hsb{q}", name=f"hsb{q}")
                        for q in range(2)
                    ]
                    hTp = [
                        psT.tile([128, 2, BL], BF16, tag=f"hTp{q}", name=f"hTp{q}")
                        for q in range(2)
                    ]
                    # ACT: gate activations for both halves first, split
                    # per gate bank so each releases its PSUM tile early
                    for q in range(2):
                        nc.scalar.activation(
                            ah[q][:, 0:512], gh[2 * q][:], AF.Sigmoid
                        )
                        nc.scalar.activation(
                            ah[q][:, 512:768], gh[2 * q + 1][:, 0:256], AF.Sigmoid
                        )
                        nc.scalar.activation(
                            ah[q][:, 768:1024], gh[2 * q + 1][:, 256:512], AF.Tanh
                        )
                    # DVE: c updates for both halves
                    for q in range(2):
                        nc.vector.tensor_mul(
                            fc[q][:], ah[q][:, 256:512], c_half[q][:]
                        )
                        nc.vector.tensor_mul(
                            ig[q][:], ah[q][:, 512:768], ah[q][:, 768:1024]
                        )
                        nc.vector.tensor_add(c_half[q][:], fc[q][:], ig[q][:])
                    # PE stream: fill work before next seeds (in-order engine)
                    if t % 2 == 0:
                        emit_xw_half()
                    if ti == 2 and ch > 0:
                        emit_linear_chunk(ch - 1)
                    if t + 1 < T:
                        gh_next = emit_seeds(t + 1)
                    # tail per half: tanh(c) -> h -> transpose -> hT copy
                    for q in range(2):
                        nc.scalar.activation(tct[q][:], c_half[q][:], AF.Tanh)
                        nc.vector.tensor_mul(h_sb[q][:], ah[q][:, 0:256], tct[q][:])
                        for kk in range(2):
                            nc.tensor.transpose(
                                hTp[q][:, kk],
                                h_sb[q][:, 128 * kk : 128 * kk + 128],
                                id32_sb[:],
                            )
                        nc.vector.tensor_copy(hT_k[q][:, ch, :, ti, :], hTp[q][:])
                    if t + 1 < T:
                        gh = gh_next

                emit_linear_chunk(TCH - 1)
            nc.sync.dma_start(outp[:], out_all[:])
    nc.compile()
    return nc


@functools.lru_cache(maxsize=1)
def _program():
    return build_nc()


def _gate_perm():
    # PyTorch gate row order: i (0:H), f (H:2H), g (2H:3H), o (3H:4H).
    # Target layout per 1024-col half q: [o f i g] x 256 covering hidden
    # units 256q:256q+256, so sigmoid spans 768 contiguous cols.
    perm = []
    for q in range(2):
        perm += list(range(3 * H + 256 * q, 3 * H + 256 * q + 256))  # o
        perm += list(range(1 * H + 256 * q, 1 * H + 256 * q + 256))  # f
        perm += list(range(0 * H + 256 * q, 0 * H + 256 * q + 256))  # i
        perm += list(range(2 * H + 256 * q, 2 * H + 256 * q + 256))  # g
    return np.asarray(perm)


def _prep_core(x, W_ih, W_hh, b_ih, b_hh, W_lin, direction, bs):
    perm = _gate_perm()
    bf16 = ml_dtypes.bfloat16
    xs = np.asarray(x)[:, bs : bs + BL, :]
    if direction == 1:
        xs = xs[::-1]
    # xT[p, ch, k, ti, b] = xs[4*ch+ti, b, 128k+p]
    xT = np.ascontiguousarray(
        xs.reshape(TCH, 4, BL, KT, 128).transpose(4, 0, 3, 1, 2)
    ).astype(bf16)
    Wp_ih = np.asarray(W_ih)[perm]  # [G4, IN]
    Wp_hh = np.asarray(W_hh)[perm]
    bp = (np.asarray(b_ih) + np.asarray(b_hh))[perm].astype(np.float32)
    wih = np.ascontiguousarray(Wp_ih.T.reshape(KT, 128, G4).transpose(1, 0, 2)).astype(
        bf16
    )
    whh = np.ascontiguousarray(Wp_hh.T.reshape(KT, 128, G4).transpose(1, 0, 2)).astype(
        bf16
    )
    biasr = np.ascontiguousarray(
        np.broadcast_to(bp.astype(bf16), (128, G4))
    )
    ones = np.ones((1, 128), dtype=bf16)
    Wl = np.asarray(W_lin)[:, direction * H : (direction + 1) * H]  # [OUT, H]
    wlin = np.ascontiguousarray(Wl.T.reshape(KT, 128, OUT).transpose(1, 0, 2)).astype(
        bf16
    )
    return {
        "xT": xT,
        "wih": wih,
        "whh": whh,
        "biasr": biasr,
        "ones": ones,
        "wlin": wlin,
        "id32": np.eye(BL, dtype=bf16),
        "id128": np.eye(128, dtype=bf16),
    }


def run_cores(inputs, trace=False):
    """Build per-core in_maps, run on 8 cores, return BassKernelResults."""
    in_maps = []
    for core in range(NCORES):
        direction = core // 4
        bs = (core % 4) * BL
        wk = "f" if direction == 0 else "b"
        in_maps.append(
            _prep_core(
                inputs["x"],
                inputs[f"W_ih_{wk}"],
                inputs[f"W_hh_{wk}"],
                inputs[f"b_ih_{wk}"],
                inputs[f"b_hh_{wk}"],
                inputs["W_lin"],
                direction,
                bs,
            )
        )
    nc = _program()
    return run_bass_kernel_spmd(nc, in_maps, list(range(NCORES)), trace=trace)


def _assemble(results, b_lin):
    # per-core outp: [128(=4 ti x 32 b), TCH, OUT] in compute-time order
    out = np.zeros((T, B, OUT), np.float32)
    for core in range(NCORES):
        direction = core // 4
        bs = (core % 4) * BL
        dev = np.asarray(results[core]["outp"], np.float32)  # [128, TCH, OUT]
        # t = 4*ch + ti, partition p = 32*ti + b
        part = (
            dev.reshape(4, BL, TCH, OUT).transpose(2, 0, 1, 3).reshape(T, BL, OUT)
        )
        if direction == 1:
            part = part[::-1]
        out[:, bs : bs + BL, :] += part
    out += np.asarray(b_lin, np.float32)[None, None, :]
    return out


def kernel(**inputs):
    res = run_cores(inputs, trace=False)
    return _assemble(res.results, inputs["b_lin"])



# revision 11
# speedup vs baseline: 1.7498x; 1.7498x over previous
"""Bidirectional LSTM Trainium2 Bass kernel — gates-transposed layout.

Problem: T=128, B=128, IN=512, H=512, OUT=512 (fp32 reference).
Sharding: data-parallel over batch + direction-parallel:
  cores 0-3: forward LSTM, batch slices 0:32, 32:64, 64:96, 96:128
  cores 4-7: backward LSTM (time-reversed x), same batch slices

Key idea vs the previous kernel: keep EVERYTHING transposed — gates,
c, h live as [feature-on-partition, batch-free] tiles. The recurrence
matmul then uses W_hh^T blocks as the STATIONARY operand and h^T
(only BL=32 columns) as the MOVING operand, so each of the 64 matmuls
per step costs N=32 rows instead of streaming the 512-wide W_hh
(N=512) — a 4x reduction in PE cycles. It also kills the per-step h
transposes entirely (h^T is what the cell update naturally produces)
and the xw seed matmuls (phase 1 computes xw^T directly INTO the same
PSUM bank the recurrence accumulates into).

Layout per step t: one PSUM bank holds gates^T [128, 16, 32] fp32,
16 gate-tiles x 32 batch. Gate-tile order (after host-side row perm):
  [i0 i1 f0 f1 o0 o1 g0 g1 | i2 i3 f2 f3 o2 o3 g2 g3]
(half h covers hidden units 256h:256h+256) so one sigmoid covers
i,i,f,f,o,o contiguously per half and one tanh covers g,g.

Per step: bias seed (4 matmuls, K=4 selection), phase-1 xw^T (64
matmuls N=32, emitted LOOKAHEAD steps early), W-MMs (64 matmuls N=32),
activations on ScalarE, c-update on VectorE, h-mul on GPSIMD writing
h^T straight into the SBUF history consumed by the next step's W-MMs
and by phase 3 (out^T = W_lin^T-blocks @ h^T, chunked, DMA'd per
chunk). Host combines: out = out_fwd + flip_t(out_bwd) + b_lin.
"""

import sys

sys.path.insert(0, "/opt/trn_rl_repo")

import functools
import os

import ml_dtypes
import numpy as np

import concourse.bass as bass
import concourse.tile as tile
from concourse import bacc, mybir
from concourse.bass_utils import run_bass_kernel_spmd

T, B, IN, H, OUT = 128, 128, 512, 512, 512
NCORES = 8
BL = B // 4  # batch per core (4 cores per direction)
G4 = 4 * H  # 2048 gate rows (transposed: gate-on-partition)
KT = IN // 128  # 4 k-tiles of 128
NGT = G4 // 128  # 16 gate tiles of 128
TCH = T // 4  # 32 column-chunks of 128 (4 steps x 32 batch)
NC_COLS = T * BL  # 4096 (t*32+b) columns

LOOKAHEAD = int(os.environ.get("LSTM_LOOKAHEAD", "4"))
RING = int(os.environ.get("LSTM_RING", "6"))  # psum gates ring (banks)

BF16 = mybir.dt.bfloat16
FP32 = mybir.dt.float32
AF = mybir.ActivationFunctionType


def build_nc(reps=1):
    nc = bacc.Bacc(None, target_bir_lowering=False)
    xT = nc.dram_tensor("xT", [128, KT, NC_COLS], BF16, kind="ExternalInput")
    wihT = nc.dram_tensor("wihT", [128, KT, G4], BF16, kind="ExternalInput")
    whhT = nc.dram_tensor("whhT", [128, KT, G4], BF16, kind="ExternalInput")
    wlinT = nc.dram_tensor("wlinT", [128, KT, OUT], BF16, kind="ExternalInput")
    biasm = nc.dram_tensor("biasm", [16, 128], BF16, kind="ExternalInput")
    sel16 = nc.dram_tensor("sel16", [16, NGT * BL], BF16, kind="ExternalInput")
    outp = nc.dram_tensor("outp", [128, 4, NC_COLS], FP32, kind="ExternalOutput")
    debug_t0 = os.environ.get("LSTM_DEBUG_T0") == "1"
    if debug_t0:
        dbg_gates = nc.dram_tensor("dbg_gates", [128, NGT, BL], FP32, kind="ExternalOutput")
        dbg_h = nc.dram_tensor("dbg_h", [128, KT, BL], FP32, kind="ExternalOutput")

    with tile.TileContext(nc) as tc:
        with (
            tc.tile_pool(name="const", bufs=1) as constp,
            tc.tile_pool(name="xring", bufs=4) as xring,
            tc.tile_pool(name="acts", bufs=3) as actsp,
            tc.tile_pool(name="tmps", bufs=2) as tmpsp,
            tc.tile_pool(name="outsb", bufs=3) as outsbp,
            tc.tile_pool(name="gates", bufs=RING, space="PSUM") as gatesp,
            tc.tile_pool(name="ps3", bufs=2, space="PSUM") as ps3,
        ):
            wih_sb = constp.tile([128, KT, G4], BF16)
            nc.sync.dma_start(wih_sb[:], wihT[:])
            biasm_sb = constp.tile([16, 128], BF16)
            nc.sync.dma_start(biasm_sb[:], biasm[:])
            sel16_sb = constp.tile([16, NGT * BL], BF16)
            nc.sync.dma_start(sel16_sb[:], sel16[:])
            whh_sb = constp.tile([128, KT, G4], BF16)
            nc.sync.dma_start(whh_sb[:], whhT[:])
            wlin_sb = constp.tile([128, KT, OUT], BF16)
            nc.sync.dma_start(wlin_sb[:], wlinT[:])
            # h^T history: [128, k-tile, t*32+b]; written per (half, step),
            # read by next step's W-MMs and by phase 3 (subtile deps).
            hT_sb = constp.tile([128, KT, NC_COLS], BF16)
            # cell state per half, [128, 2 k-tiles, 32] fp32
            c_half = [constp.tile([128, 2, BL], FP32, name=f"c{q}") for q in range(2)]

            for _rep in range(reps):
                for q in range(2):
                    nc.vector.memset(c_half[q][:], 0.0)
                banks = {}
                xch_tiles = {}

                def ensure_xchunk(ch):
                    if ch not in xch_tiles:
                        xt = xring.tile([128, KT, 128], BF16, tag="xch", name="xch")
                        nc.sync.dma_start(xt[:], xT[:, :, 128 * ch : 128 * ch + 128])
                        xch_tiles[ch] = xt
                    return xch_tiles[ch]

                def emit_ph1(s):
                    ch, ti = s // 4, s % 4
                    xt = ensure_xchunk(ch)
                    bank = gatesp.tile([128, NGT, BL], FP32, tag="bank", name="bank")
                    banks[s] = bank
                    # single whole-bank seed: start=True zeroes the entire
                    # PSUM bank, so there must be exactly one start per bank
                    nc.tensor.matmul(
                        bank[:],
                        biasm_sb[:],
                        sel16_sb[:],
                        start=True,
                        stop=False,
                        skip_group_check=True,
                    )
                    for k in range(KT):
                        for gt in range(NGT):
                            nc.tensor.matmul(
                                bank[:, gt, :],
                                wih_sb[:, k, 128 * gt : 128 * gt + 128],
                                xt[:, k, 32 * ti : 32 * ti + 32],
                                start=False,
                                stop=(s == 0 and k == KT - 1),
                                skip_group_check=True,
                            )

                def emit_wmm(t):
                    bank = banks[t]
                    cols = slice(32 * (t - 1), 32 * (t - 1) + 32)
                    for k in range(KT):
                        # half-0 gate tiles first so their groups close
                        # before half-1's within the k=3 block
                        for gt in range(NGT):
                            nc.tensor.matmul(
                                bank[:, gt, :],
                                whh_sb[:, k, 128 * gt : 128 * gt + 128],
                                hT_sb[:, k, cols],
                                start=False,
                                stop=(k == KT - 1),
                                skip_group_check=True,
                            )

                def emit_cell(t):
                    bank = banks.pop(t)
                    if debug_t0 and t == 0:
                        gsb = constp.tile([128, NGT, BL], FP32, name="gsb")
                        nc.vector.tensor_copy(gsb[:], bank[:])
                        nc.sync.dma_start(dbg_gates[:], gsb[:])
                    ah, ag, tct = [], [], []
                    for q in range(2):
                        ah.append(
                            actsp.tile([128, 6, BL], BF16, tag=f"ah{q}", name=f"ah{q}")
                        )
                        ag.append(
                            actsp.tile([128, 2, BL], BF16, tag=f"ag{q}", name=f"ag{q}")
                        )
                        tct.append(
                            actsp.tile([128, 2, BL], BF16, tag=f"tc{q}", name=f"tc{q}")
                        )
                        fc = tmpsp.tile([128, 2, BL], FP32, tag=f"fc{q}", name=f"fc{q}")
                        ig = tmpsp.tile([128, 2, BL], FP32, tag=f"ig{q}", name=f"ig{q}")
                        nc.scalar.activation(
                            ah[q][:], bank[:, 8 * q : 8 * q + 6, :], AF.Sigmoid
                        )
                        nc.scalar.activation(
                            ag[q][:], bank[:, 8 * q + 6 : 8 * q + 8, :], AF.Tanh
                        )
                        nc.vector.tensor_mul(fc[:], ah[q][:, 2:4, :], c_half[q][:])
                        nc.vector.tensor_mul(ig[:], ah[q][:, 0:2, :], ag[q][:])
                        nc.vector.tensor_add(c_half[q][:], fc[:], ig[:])
                    for q in range(2):
                        nc.scalar.activation(tct[q][:], c_half[q][:], AF.Tanh)
                        nc.gpsimd.tensor_mul(
                            hT_sb[:, 2 * q : 2 * q + 2, 32 * t : 32 * t + 32],
                            ah[q][:, 4:6, :],
                            tct[q][:],
                        )

                def emit_ph3(ch):
                    po = ps3.tile([128, 4, 128], FP32, tag="po", name="po")
                    cols = slice(128 * ch, 128 * ch + 128)
                    for ot in range(4):
                        for k in range(KT):
                            nc.tensor.matmul(
                                po[:, ot, :],
                                wlin_sb[:, k, 128 * ot : 128 * ot + 128],
                                hT_sb[:, k, cols],
                                start=(ot == 0 and k == 0),
                                stop=(k == KT - 1),
                                skip_group_check=True,
                            )
                    ob = outsbp.tile([128, 4, 128], FP32, tag="ob", name="ob")
                    nc.vector.tensor_copy(ob[:], po[:])
                    nc.sync.dma_start(outp[:, :, cols], ob[:])

                for s in range(LOOKAHEAD):
                    emit_ph1(s)
                for t in range(T):
                    if debug_t0 and t == 1:
                        hsb = constp.tile([128, KT, BL], FP32, name="hsb")
                        nc.vector.tensor_copy(hsb[:], hT_sb[:, :, 0:BL])
                        nc.sync.dma_start(dbg_h[:], hsb[:])
                    if t > 0:
                        emit_wmm(t)
                    emit_cell(t)
                    if t + LOOKAHEAD < T:
                        emit_ph1(t + LOOKAHEAD)
                    if t % 4 == 2 and t >= 4:
                        emit_ph3(t // 4 - 1)
                emit_ph3(TCH - 1)
    nc.compile()
    return nc


@functools.lru_cache(maxsize=1)
def _program():
    return build_nc()


def _gate_perm():
    # PyTorch gate row order: i (0:H), f (H:2H), g (2H:3H), o (3H:4H).
    # Target gate-tile order per half h: [i(2h) i(2h+1) f.. f.. o.. o.. g.. g..]
    # where tile j of gate X = rows X_off + 128j : +128 (hidden units 128j:..).
    off = {"i": 0, "f": H, "g": 2 * H, "o": 3 * H}
    perm = []
    for h in range(2):
        for gate in ("i", "f", "o", "g"):
            for j in (2 * h, 2 * h + 1):
                perm += list(range(off[gate] + 128 * j, off[gate] + 128 * j + 128))
    # reorder within half: built i,i,f,f,o,o,g,g — matches kernel slices
    return np.asarray(perm)


def _prep_core(x, W_ih, W_hh, b_ih, b_hh, W_lin, direction, bs):
    perm = _gate_perm()
    bf16 = ml_dtypes.bfloat16
    xs = np.asarray(x)[:, bs : bs + BL, :]
    if direction == 1:
        xs = xs[::-1]
    # xT[p, k, t*32+b] = xs[t, b, 128k+p]
    xTl = np.ascontiguousarray(
        xs.reshape(T, BL, KT, 128).transpose(3, 2, 0, 1).reshape(128, KT, NC_COLS)
    ).astype(bf16)
    Wp_ih = np.asarray(W_ih)[perm]  # [G4, IN]
    Wp_hh = np.asarray(W_hh)[perm]  # [G4, H]
    wihT = np.ascontiguousarray(
        Wp_ih.T.reshape(KT, 128, G4).transpose(1, 0, 2)
    ).astype(bf16)
    whhT = np.ascontiguousarray(
        Wp_hh.T.reshape(KT, 128, G4).transpose(1, 0, 2)
    ).astype(bf16)
    bp = (np.asarray(b_ih) + np.asarray(b_hh))[perm].astype(np.float32)
    # biasm[k, p] = bp[128k + p]; seed matmul: out[p, gt, b] = bias[128gt+p]
    biasm = np.ascontiguousarray(bp.reshape(16, 128)).astype(bf16)
    sel16 = np.ascontiguousarray(np.repeat(np.eye(16, dtype=bf16), BL, axis=1))
    Wl = np.asarray(W_lin)[:, direction * H : (direction + 1) * H]  # [OUT, H]
    wlinT = np.ascontiguousarray(
        Wl.T.reshape(KT, 128, OUT).transpose(1, 0, 2)
    ).astype(bf16)
    return {
        "xT": xTl,
        "wihT": wihT,
        "whhT": whhT,
        "wlinT": wlinT,
        "biasm": biasm,
        "sel16": sel16,
    }


def run_cores(inputs, trace=False):
    """Build per-core in_maps, run on 8 cores, return BassKernelResults."""
    in_maps = []
    for core in range(NCORES):
        direction = core // 4
        bs = (core % 4) * BL
        wk = "f" if direction == 0 else "b"
        in_maps.append(
            _prep_core(
                inputs["x"],
                inputs[f"W_ih_{wk}"],
                inputs[f"W_hh_{wk}"],
                inputs[f"b_ih_{wk}"],
                inputs[f"b_hh_{wk}"],
                inputs["W_lin"],
                direction,
                bs,
            )
        )
    nc = _program()
    return run_bass_kernel_spmd(nc, in_maps, list(range(NCORES)), trace=trace)


def _assemble(results, b_lin):
    # per-core outp: [128, 4, T*BL]; part[t, b, 128*ot+p] = outp[p, ot, 32t+b]
    out = np.zeros((T, B, OUT), np.float32)
    for core in range(NCORES):
        direction = core // 4
        bs = (core % 4) * BL
        dev = np.asarray(results[core]["outp"], np.float32)  # [128, 4, 4096]
        part = dev.reshape(128, 4, T, BL).transpose(2, 3, 1, 0).reshape(T, BL, OUT)
        if direction == 1:
            part = part[::-1]
        out[:, bs : bs + BL, :] += part
    out += np.asarray(b_lin, np.float32)[None, None, :]
    return out


def kernel(**inputs):
    res = run_cores(inputs, trace=False)
    return _assemble(res.results, inputs["b_lin"])


# revision 14
# speedup vs baseline: 1.7811x; 1.0179x over previous
"""Bidirectional LSTM Trainium2 Bass kernel — gates-transposed layout.

Problem: T=128, B=128, IN=512, H=512, OUT=512 (fp32 reference).
Sharding: data-parallel over batch + direction-parallel:
  cores 0-3: forward LSTM, batch slices 0:32, 32:64, 64:96, 96:128
  cores 4-7: backward LSTM (time-reversed x), same batch slices

Key idea vs the previous kernel: keep EVERYTHING transposed — gates,
c, h live as [feature-on-partition, batch-free] tiles. The recurrence
matmul then uses W_hh^T blocks as the STATIONARY operand and h^T
(only BL=32 columns) as the MOVING operand, so each of the 64 matmuls
per step costs N=32 rows instead of streaming the 512-wide W_hh
(N=512) — a 4x reduction in PE cycles. It also kills the per-step h
transposes entirely (h^T is what the cell update naturally produces)
and the xw seed matmuls (phase 1 computes xw^T directly INTO the same
PSUM bank the recurrence accumulates into).

Layout per step t: one PSUM bank holds gates^T [128, 16, 32] fp32,
16 gate-tiles x 32 batch. Gate-tile order (after host-side row perm):
  [i0 i1 f0 f1 o0 o1 g0 g1 | i2 i3 f2 f3 o2 o3 g2 g3]
(half h covers hidden units 256h:256h+256) so one sigmoid covers
i,i,f,f,o,o contiguously per half and one tanh covers g,g.

Per step: bias seed (4 matmuls, K=4 selection), phase-1 xw^T (64
matmuls N=32, emitted LOOKAHEAD steps early), W-MMs (64 matmuls N=32),
activations on ScalarE, c-update on VectorE, h-mul on GPSIMD writing
h^T straight into the SBUF history consumed by the next step's W-MMs
and by phase 3 (out^T = W_lin^T-blocks @ h^T, chunked, DMA'd per
chunk). Host combines: out = out_fwd + flip_t(out_bwd) + b_lin.
"""

import sys

sys.path.insert(0, "/opt/trn_rl_repo")

import functools
import os

import ml_dtypes
import numpy as np

import concourse.bass as bass
import concourse.tile as tile
from concourse import bacc, mybir
from concourse.bass_utils import run_bass_kernel_spmd

T, B, IN, H, OUT = 128, 128, 512, 512, 512
NCORES = 8
BL = B // 4  # batch per core (4 cores per direction)
G4 = 4 * H  # 2048 gate rows (transposed: gate-on-partition)
KT = IN // 128  # 4 k-tiles of 128
NGT = G4 // 128  # 16 gate tiles of 128
TCH = T // 4  # 32 column-chunks of 128 (4 steps x 32 batch)
NC_COLS = T * BL  # 4096 (t*32+b) columns

LOOKAHEAD = int(os.environ.get("LSTM_LOOKAHEAD", "4"))
RING = int(os.environ.get("LSTM_RING", "6"))  # psum gates ring (banks)

BF16 = mybir.dt.bfloat16
FP32 = mybir.dt.float32
AF = mybir.ActivationFunctionType


def build_nc(reps=1):
    nc = bacc.Bacc(None, target_bir_lowering=False)
    xT = nc.dram_tensor("xT", [128, KT, NC_COLS], BF16, kind="ExternalInput")
    wihT = nc.dram_tensor("wihT", [128, KT, G4], BF16, kind="ExternalInput")
    whhT = nc.dram_tensor("whhT", [128, KT, G4], BF16, kind="ExternalInput")
    wlinT = nc.dram_tensor("wlinT", [128, KT, OUT], BF16, kind="ExternalInput")
    biasm = nc.dram_tensor("biasm", [16, 128], BF16, kind="ExternalInput")
    sel16 = nc.dram_tensor("sel16", [16, NGT * BL], BF16, kind="ExternalInput")
    outp = nc.dram_tensor("outp", [128, 4, NC_COLS], FP32, kind="ExternalOutput")
    debug_t0 = os.environ.get("LSTM_DEBUG_T0") == "1"
    if debug_t0:
        dbg_gates = nc.dram_tensor("dbg_gates", [128, NGT, BL], FP32, kind="ExternalOutput")
        dbg_h = nc.dram_tensor("dbg_h", [128, KT, BL], FP32, kind="ExternalOutput")

    with tile.TileContext(nc) as tc:
        with (
            tc.tile_pool(name="const", bufs=1) as constp,
            tc.tile_pool(name="xring", bufs=4) as xring,
            tc.tile_pool(name="acts", bufs=3) as actsp,
            tc.tile_pool(name="tmps", bufs=2) as tmpsp,
            tc.tile_pool(name="outsb", bufs=3) as outsbp,
            tc.tile_pool(name="gates", bufs=RING, space="PSUM") as gatesp,
            tc.tile_pool(name="ps3", bufs=2, space="PSUM") as ps3,
        ):
            wih_sb = constp.tile([128, KT, G4], BF16)
            nc.sync.dma_start(wih_sb[:], wihT[:])
            biasm_sb = constp.tile([16, 128], BF16)
            nc.sync.dma_start(biasm_sb[:], biasm[:])
            sel16_sb = constp.tile([16, NGT * BL], BF16)
            nc.sync.dma_start(sel16_sb[:], sel16[:])
            whh_sb = constp.tile([128, KT, G4], BF16)
            nc.sync.dma_start(whh_sb[:], whhT[:])
            wlin_sb = constp.tile([128, KT, OUT], BF16)
            nc.sync.dma_start(wlin_sb[:], wlinT[:])
            # h^T history: [128, k-tile, t*32+b]; written per (half, step),
            # read by next step's W-MMs and by phase 3 (subtile deps).
            hT_sb = constp.tile([128, KT, NC_COLS], BF16)
            # cell state per half, [128, 2 k-tiles, 32] fp32
            c_half = [constp.tile([128, 2, BL], FP32, name=f"c{q}") for q in range(2)]

            for _rep in range(reps):
                for q in range(2):
                    nc.vector.memset(c_half[q][:], 0.0)
                banks = {}
                xch_tiles = {}

                def ensure_xchunk(ch):
                    if ch not in xch_tiles:
                        xt = xring.tile([128, KT, 128], BF16, tag="xch", name="xch")
                        nc.sync.dma_start(xt[:], xT[:, :, 128 * ch : 128 * ch + 128])
                        xch_tiles[ch] = xt
                    return xch_tiles[ch]

                def emit_ph1(s):
                    ch, ti = s // 4, s % 4
                    xt = ensure_xchunk(ch)
                    bank = gatesp.tile([128, NGT, BL], FP32, tag="bank", name="bank")
                    banks[s] = bank
                    # single whole-bank seed: start=True zeroes the entire
                    # PSUM bank, so there must be exactly one start per bank
                    nc.tensor.matmul(
                        bank[:],
                        biasm_sb[:],
                        sel16_sb[:],
                        start=True,
                        stop=False,
                        skip_group_check=True,
                    )
                    for k in range(KT):
                        for gt in range(NGT):
                            nc.tensor.matmul(
                                bank[:, gt, :],
                                wih_sb[:, k, 128 * gt : 128 * gt + 128],
                                xt[:, k, 32 * ti : 32 * ti + 32],
                                start=False,
                                stop=(s == 0 and k == KT - 1),
                                skip_group_check=True,
                            )

                def emit_wmm(t):
                    bank = banks[t]
                    cols = slice(32 * (t - 1), 32 * (t - 1) + 32)
                    # k0/k1 blocks (need hT half-0) first; then k2/k3 with
                    # half-1's OWN gate tiles first so sig_h1 — the critical
                    # chain — unblocks as early as possible after hT half-1.
                    for k, gts in (
                        (0, range(NGT)),
                        (1, range(NGT)),
                        (2, range(8, NGT)),
                        (3, range(8, NGT)),
                        (2, range(8)),
                        (3, range(8)),
                    ):
                        for gt in gts:
                            nc.tensor.matmul(
                                bank[:, gt, :],
                                whh_sb[:, k, 128 * gt : 128 * gt + 128],
                                hT_sb[:, k, cols],
                                start=False,
                                stop=(k == KT - 1),
                                skip_group_check=True,
                            )

                def emit_cell(t):
                    bank = banks.pop(t)
                    if debug_t0 and t == 0:
                        gsb = constp.tile([128, NGT, BL], FP32, name="gsb")
                        nc.vector.tensor_copy(gsb[:], bank[:])
                        nc.sync.dma_start(dbg_gates[:], gsb[:])
                    ah, ag, tct = [], [], []
                    for q in range(2):
                        ah.append(
                            actsp.tile([128, 6, BL], BF16, tag=f"ah{q}", name=f"ah{q}")
                        )
                        ag.append(
                            actsp.tile([128, 2, BL], BF16, tag=f"ag{q}", name=f"ag{q}")
                        )
                        tct.append(
                            actsp.tile([128, 2, BL], BF16, tag=f"tc{q}", name=f"tc{q}")
                        )
                        fc = tmpsp.tile([128, 2, BL], FP32, tag=f"fc{q}", name=f"fc{q}")
                        ig = tmpsp.tile([128, 2, BL], FP32, tag=f"ig{q}", name=f"ig{q}")
                        nc.scalar.activation(
                            ah[q][:], bank[:, 8 * q : 8 * q + 6, :], AF.Sigmoid
                        )
                        nc.scalar.activation(
                            ag[q][:], bank[:, 8 * q + 6 : 8 * q + 8, :], AF.Tanh
                        )
                        # fc on GPSIMD in parallel with ig on DVE
                        nc.gpsimd.tensor_mul(fc[:], ah[q][:, 2:4, :], c_half[q][:])
                        nc.vector.tensor_mul(ig[:], ah[q][:, 0:2, :], ag[q][:])
                        nc.vector.tensor_add(c_half[q][:], fc[:], ig[:])
                    for q in range(2):
                        nc.scalar.activation(tct[q][:], c_half[q][:], AF.Tanh)
                        # h-mul: half-0 on GPSIMD (has slack), half-1 on DVE
                        # (critical chain — avoids the Pool launch latency)
                        eng = nc.gpsimd if q == 0 else nc.vector
                        eng.tensor_mul(
                            hT_sb[:, 2 * q : 2 * q + 2, 32 * t : 32 * t + 32],
                            ah[q][:, 4:6, :],
                            tct[q][:],
                        )

                def emit_ph3(ch):
                    po = ps3.tile([128, 4, 128], FP32, tag="po", name="po")
                    cols = slice(128 * ch, 128 * ch + 128)
                    for ot in range(4):
                        for k in range(KT):
                            nc.tensor.matmul(
                                po[:, ot, :],
                                wlin_sb[:, k, 128 * ot : 128 * ot + 128],
                                hT_sb[:, k, cols],
                                start=(ot == 0 and k == 0),
                                stop=(k == KT - 1),
                                skip_group_check=True,
                            )
                    ob = outsbp.tile([128, 4, 128], FP32, tag="ob", name="ob")
                    nc.gpsimd.tensor_copy(ob[:], po[:])
                    nc.sync.dma_start(outp[:, :, cols], ob[:])

                for s in range(LOOKAHEAD):
                    emit_ph1(s)
                for t in range(T):
                    if debug_t0 and t == 1:
                        hsb = constp.tile([128, KT, BL], FP32, name="hsb")
                        nc.vector.tensor_copy(hsb[:], hT_sb[:, :, 0:BL])
                        nc.sync.dma_start(dbg_h[:], hsb[:])
                    if t > 0:
                        emit_wmm(t)
                    emit_cell(t)
                    if t + LOOKAHEAD < T:
                        emit_ph1(t + LOOKAHEAD)
                    if t % 4 == 2 and t >= 4:
                        emit_ph3(t // 4 - 1)
                emit_ph3(TCH - 1)
    nc.compile()
    return nc


@functools.lru_cache(maxsize=1)
def _program():
    return build_nc()


def _gate_perm():
    # PyTorch gate row order: i (0:H), f (H:2H), g (2H:3H), o (3H:4H).
    # Target gate-tile order per half h: [i(2h) i(2h+1) f.. f.. o.. o.. g.. g..]
    # where tile j of gate X = rows X_off + 128j : +128 (hidden units 128j:..).
    off = {"i": 0, "f": H, "g": 2 * H, "o": 3 * H}
    perm = []
    for h in range(2):
        for gate in ("i", "f", "o", "g"):
            for j in (2 * h, 2 * h + 1):
                perm += list(range(off[gate] + 128 * j, off[gate] + 128 * j + 128))
    # reorder within half: built i,i,f,f,o,o,g,g — matches kernel slices
    return np.asarray(perm)


def _prep_core(x, W_ih, W_hh, b_ih, b_hh, W_lin, direction, bs):
    perm = _gate_perm()
    bf16 = ml_dtypes.bfloat16
    xs = np.asarray(x)[:, bs : bs + BL, :]
    if direction == 1:
        xs = xs[::-1]
    # xT[p, k, t*32+b] = xs[t, b, 128k+p]
    xTl = np.ascontiguousarray(
        xs.reshape(T, BL, KT, 128).transpose(3, 2, 0, 1).reshape(128, KT, NC_COLS)
    ).astype(bf16)
    Wp_ih = np.asarray(W_ih)[perm]  # [G4, IN]
    Wp_hh = np.asarray(W_hh)[perm]  # [G4, H]
    wihT = np.ascontiguousarray(
        Wp_ih.T.reshape(KT, 128, G4).transpose(1, 0, 2)
    ).astype(bf16)
    whhT = np.ascontiguousarray(
        Wp_hh.T.reshape(KT, 128, G4).transpose(1, 0, 2)
    ).astype(bf16)
    bp = (np.asarray(b_ih) + np.asarray(b_hh))[perm].astype(np.float32)
    # biasm[k, p] = bp[128k + p]; seed matmul: out[p, gt, b] = bias[128gt+p]
    biasm = np.ascontiguousarray(bp.reshape(16, 128)).astype(bf16)
    sel16 = np.ascontiguousarray(np.repeat(np.eye(16, dtype=bf16), BL, axis=1))
    Wl = np.asarray(W_lin)[:, direction * H : (direction + 1) * H]  # [OUT, H]
    wlinT = np.ascontiguousarray(
        Wl.T.reshape(KT, 128, OUT).transpose(1, 0, 2)
    ).astype(bf16)
    return {
        "xT": xTl,
        "wihT": wihT,
        "whhT": whhT,
        "wlinT": wlinT,
        "biasm": biasm,
        "sel16": sel16,
    }


def run_cores(inputs, trace=False):
    """Build per-core in_maps, run on 8 cores, return BassKernelResults."""
    in_maps = []
    for core in range(NCORES):
        direction = core // 4
        bs = (core % 4) * BL
        wk = "f" if direction == 0 else "b"
        in_maps.append(
            _prep_core(
                inputs["x"],
                inputs[f"W_ih_{wk}"],
                inputs[f"W_hh_{wk}"],
                inputs[f"b_ih_{wk}"],
                inputs[f"b_hh_{wk}"],
                inputs["W_lin"],
                direction,
                bs,
            )
        )
    nc = _program()
    return run_bass_kernel_spmd(nc, in_maps, list(range(NCORES)), trace=trace)


def _assemble(results, b_lin):
    # per-core outp: [128, 4, T*BL]; part[t, b, 128*ot+p] = outp[p, ot, 32t+b]
    out = np.zeros((T, B, OUT), np.float32)
    for core in range(NCORES):
        direction = core // 4
        bs = (core % 4) * BL
        dev = np.asarray(results[core]["outp"], np.float32)  # [128, 4, 4096]
        part = dev.reshape(128, 4, T, BL).transpose(2, 3, 1, 0).reshape(T, BL, OUT)
        if direction == 1:
            part = part[::-1]
        out[:, bs : bs + BL, :] += part
    out += np.asarray(b_lin, np.float32)[None, None, :]
    return out


def kernel(**inputs):
    res = run_cores(inputs, trace=False)
    return _assemble(res.results, inputs["b_lin"])
